# revision 1
# baseline (speedup 1.0000x reference)
"""Trainium2 Bass kernel: 2-layer GCN (GCNConv -> ReLU -> GCNConv -> Linear).

Strategy (8 NeuronCores, SPMD):
  - Destination-node sharding: core k owns nodes [k*6250, (k+1)*6250).
  - 3 launches with host-side exchange of the (small) activation tables:
      L1: H1 = X @ W1            (row-sharded dense matmul)
      L2: MP1 + bias + ReLU, then @ W2 -> H2   (message passing via dma_gather
          + PE segment-reduction with host-built one-hot*norm weight blocks)
      L3: MP2 + bias, then @ Wp + bp -> out
  - Message passing: edges sorted by destination; gathered source rows land on
    partitions (edge position mod 128); a [128, M] one-hot-times-norm block
    matrix (lhsT) contracts 128 edges into the destination rows of a PSUM tile.
    PSUM accumulates across chunks; a bias matmul (identity x replicated-bias)
    initializes every row first.
  - int16 gather indices => table split in two halves (cores 0-3 / 4-7).
  - All matmul operands bf16 (fp32 PSUM accumulation); final output fp32.
"""

import os
from contextlib import ExitStack
from dataclasses import dataclass, field

import numpy as np
import ml_dtypes

BF16 = ml_dtypes.bfloat16
FP32 = np.float32


# ---------------------------------------------------------------- config

@dataclass
class Cfg:
    N: int = 50000
    IN_DIM: int = 512
    HID: int = 256
    OUT: int = 128
    NCORES: int = 8
    GC: int = 32          # chunks per gather (4096 idxs; needs single_packet=False)

    ND: int = field(init=False)
    NTILES: int = field(init=False)
    NP: int = field(init=False)
    TROWS: int = field(init=False)
    HALFROWS: int = field(init=False)
    SRC_SPLIT: int = field(init=False)

    def __post_init__(self):
        self.ND = self.N // self.NCORES
        self.NTILES = (self.ND + 127) // 128
        self.NP = self.NTILES * 128
        self.TROWS = self.NCORES * self.NP
        self.HALFROWS = self.TROWS // 2
        self.SRC_SPLIT = (self.NCORES // 2) * self.ND
        assert self.HALFROWS <= 32768, "int16 gather index limit"


# ---------------------------------------------------------------- planner

class Plan:
    """Static (cross-core identical) geometry + per-core data arrays."""

    def __init__(self, cfg: Cfg, edge_index, edge_weight):
        self.cfg = cfg
        N, ND, NP, NT = cfg.N, cfg.ND, cfg.NP, cfg.NTILES
        NC = cfg.NCORES

        # --- gcn_norm with self loops (host: O(E) index/weight preprocessing)
        row = np.concatenate([np.asarray(edge_index[0], np.int64),
                              np.arange(N, dtype=np.int64)])
        col = np.concatenate([np.asarray(edge_index[1], np.int64),
                              np.arange(N, dtype=np.int64)])
        w = np.concatenate([np.asarray(edge_weight, np.float64),
                            np.ones(N, np.float64)])
        deg = np.zeros(N, np.float64)
        np.add.at(deg, col, w)
        dinv = np.where(deg > 0, 1.0 / np.sqrt(deg), 0.0)
        nrm = (dinv[row] * w * dinv[col]).astype(np.float32)

        # --- global degree-sorted serpentine node->(core, lane) assignment:
        # every core gets a near-identical degree profile, so the cross-core
        # max padding of the static chunk geometry nearly vanishes.
        degi = np.bincount(col, minlength=N)
        ranks = np.argsort(-degi, kind="stable")    # rank r -> node
        r = np.arange(N)
        blk = r // NC
        corepos = np.where(blk % 2 == 0, r % NC, NC - 1 - (r % NC))
        lane_r = blk
        lane_global = np.empty(N, np.int64)        # node -> core*NP + lane
        lane_global[ranks] = corepos * NP + lane_r
        self.nodes = []                             # per core: lane -> node id
        for k in range(NC):
            nk = np.empty(ND, np.int64)
            sel = corepos == k
            nk[lane_r[sel]] = ranks[sel]
            self.nodes.append(nk)

        # self loops handled densely (tables are assignment-ordered); their
        # weight is dinv^2 * 1.0
        self.selfw = []
        for k in range(NC):
            sw = np.zeros((128, NT), np.float32)
            lanes = np.arange(ND)
            vals = (dinv[self.nodes[k]] ** 2).astype(np.float32)
            sw[lanes % 128, lanes // 128] = vals
            self.selfw.append(sw)

        # drop only the APPENDED self-loop block (original (u,u) edges stay)
        ne = len(row) - N
        row, col, nrm = row[:ne], col[:ne], nrm[:ne]

        trow2 = lane_global[row]                    # table row of the source
        half = (trow2 >= cfg.HALFROWS).astype(np.int64)
        idx2 = np.where(half == 0, trow2, trow2 - cfg.HALFROWS)
        assert idx2.min() >= 0 and idx2.max() < cfg.HALFROWS

        dst_core = lane_global[col] // NP
        dlane = lane_global[col] % NP
        dtile = dlane // 128

        order = np.lexsort((dlane, half, dtile, dst_core))
        so_core = dst_core[order]
        so_tile = dtile[order]
        so_half = half[order]
        so_lane = (dlane - dtile * 128)[order]
        so_i2 = idx2[order]
        so_w = nrm[order]

        # edges per (core, tile, half)
        key = (so_core * NT + so_tile) * 2 + so_half
        cnt = np.bincount(key, minlength=NC * NT * 2).reshape(NC, NT, 2)
        Cch = -(-cnt // 128)                         # ceil chunks per seg
        self.CH = Cch.max(axis=0)                    # [NT, 2] static
        # stream chunk bases per (tile, half)
        self.abase = np.concatenate([[0], np.cumsum(self.CH[:, 0])])  # [NT+1]
        self.bbase = np.concatenate([[0], np.cumsum(self.CH[:, 1])])
        self.totA = int(self.abase[-1])
        self.totB = int(self.bbase[-1])
        SA, SB = self.totA * 128, self.totB * 128

        # edge position within its padded stream
        # rank within segment:
        seg_start_sorted = np.concatenate([[0], np.cumsum(np.bincount(
            key, minlength=NC * NT * 2))])[:-1]
        rank = np.arange(len(key)) - seg_start_sorted[key]
        base_chunks = np.where(so_half == 0,
                               self.abase[so_tile],
                               self.bbase[so_tile])
        pos = base_chunks * 128 + rank               # position in its stream
        chunk = base_chunks + rank // 128            # stream chunk index
        lanepos = pos % 128

        # --- chunk windows (cross-core): base lane / M per (half, chunk)
        self.baseM = []
        for h, tot in ((0, self.totA), (1, self.totB)):
            m = so_half == h
            mn = np.full(tot, 128, np.int64)
            mx = np.full(tot, -1, np.int64)
            np.minimum.at(mn, chunk[m], so_lane[m])
            np.maximum.at(mx, chunk[m], so_lane[m])
            empty = mx < 0
            mn[empty] = 0
            # Legal matmul out windows: base 0 (M<=128), base 32 (M<=32),
            # base 64 (M<=64).  Slab window starts at min(32*(mn//32), 64).
            mn = np.minimum((mn // 32) * 32, 64)
            M = np.where(empty, 0, mx - mn + 1)
            self.baseM.append((mn, M))

        # matmul pieces per chunk: slots with bases (0, 32, 64); lanes >= 64
        # all go to the base-64 slot (M<=64 there, legal)
        self.pieces = []
        for h, tot in ((0, self.totA), (1, self.totB)):
            m = so_half == h
            slot = np.minimum(so_lane[m] // 32, 2)
            key2 = chunk[m] * 3 + slot
            mx2 = np.full(max(tot, 1) * 3, -1, np.int64)
            np.maximum.at(mx2, key2, so_lane[m])
            mx2 = mx2.reshape(-1, 3)[:tot]
            Ms = np.where(mx2 >= 0, mx2 - np.array([0, 32, 64]) + 1, 0)
            self.pieces.append(Ms)

        # consumption order (tile: A chunks then B chunks) -> slab offsets
        self.slab_off = [np.zeros(self.totA, np.int64),
                         np.zeros(self.totB, np.int64)]
        off = 0
        for t in range(NT):
            for h, base in ((0, self.abase), (1, self.bbase)):
                for j in range(int(base[t]), int(base[t + 1])):
                    self.slab_off[h][j] = off
                    off += int(self.baseM[h][1][j])
        self.SLAB = max(off, 1)

        # --- per-core arrays
        self.idxs = []   # (idxA, idxB) wrapped int16 [128, S/16]
        self.wslab = []  # [128, SLAB] bf16
        for k in range(NC):
            m = so_core == k
            kh, kpos, kchunk, klp = so_half[m], pos[m], chunk[m], lanepos[m]
            ki2, kw, klane = so_i2[m], so_w[m], so_lane[m]

            arrs = []
            for h, S in ((0, SA), (1, SB)):
                hm = kh == h
                lin = np.zeros(S, np.int16)
                lin[kpos[hm]] = ki2[hm].astype(np.int16)
                arrs.append(self._wrap16(lin))
            self.idxs.append((arrs[0], arrs[1]))

            slab = np.zeros((128, self.SLAB), np.float32)
            colw = self.slab_off[0] - self.baseM[0][0]
            colwB = self.slab_off[1] - self.baseM[1][0]
            hm = kh == 0
            slab[klp[hm], kchunk[hm] * 0 + colw[kchunk[hm]] + klane[hm]] = kw[hm]
            hm = kh == 1
            slab[klp[hm], colwB[kchunk[hm]] + klane[hm]] = kw[hm]
            self.wslab.append(slab.astype(BF16))

    @staticmethod
    def _wrap16(lin):
        # position i lives at [i % 16, i // 16]; replicated to 128 partitions
        w = lin.reshape(-1, 16).T.copy()
        return np.tile(w, (8, 1))


# ---------------------------------------------------------------- bass builders

def _build_l1(cfg: Cfg):
    import concourse.bacc as bacc
    import concourse.mybir as mybir
    import concourse.tile as tile

    dt = mybir.dt
    nc = bacc.Bacc(None, target_bir_lowering=False, num_swdge_queues=4)
    KCH = cfg.IN_DIM // 128
    xt = nc.dram_tensor("xt", [128, KCH * cfg.NP], dt.bfloat16, kind="ExternalInput")
    w1 = nc.dram_tensor("w1", [128, KCH * cfg.HID], dt.bfloat16, kind="ExternalInput")
    h1 = nc.dram_tensor("h1", [cfg.NP, cfg.HID], dt.bfloat16, kind="ExternalOutput")

    with tile.TileContext(nc) as tc, ExitStack() as ctx:
        consts = ctx.enter_context(tc.tile_pool(name="consts", bufs=1))
        outs = ctx.enter_context(tc.tile_pool(name="outs", bufs=3))
        psum = ctx.enter_context(tc.tile_pool(name="psum", bufs=2, space="PSUM"))

        xt_sb = consts.tile([128, KCH * cfg.NP], dt.bfloat16, tag="xt")
        nc.sync.dma_start(xt_sb[:], xt[:])
        w1_sb = consts.tile([128, KCH * cfg.HID], dt.bfloat16, tag="w1")
        nc.sync.dma_start(w1_sb[:], w1[:])

        for t in range(cfg.NTILES):
            ps = psum.tile([128, cfg.HID], dt.float32)
            for c in range(KCH):
                nc.tensor.matmul(
                    ps[:],
                    xt_sb[:, c * cfg.NP + t * 128: c * cfg.NP + (t + 1) * 128],
                    w1_sb[:, c * cfg.HID:(c + 1) * cfg.HID],
                    start=(c == 0), stop=(c == KCH - 1),
                )
            o = outs.tile([128, cfg.HID], dt.bfloat16)
            nc.scalar.activation(o[:], ps[:], mybir.ActivationFunctionType.Copy)
            nc.sync.dma_start(h1[t * 128:(t + 1) * 128, :], o[:])
    nc.finalize()
    return nc


def _build_mp(cfg: Cfg, plan: Plan, layer2: bool):
    """layer2: MP1 + ReLU + @W2 -> H2 (bf16). else: MP2 + @Wp + bp -> y (f32)."""
    import concourse.bacc as bacc
    import concourse.mybir as mybir
    import concourse.tile as tile

    dt = mybir.dt
    F = cfg.HID if layer2 else cfg.OUT           # table feature width
    FCH = F // 128
    nc = bacc.Bacc(None, target_bir_lowering=False, num_swdge_queues=4)

    tab = nc.dram_tensor("tab", [cfg.TROWS, F], dt.bfloat16, kind="ExternalInput")
    tabself = nc.dram_tensor("tabself", [cfg.NP, F], dt.bfloat16,
                             kind="ExternalInput")
    selfw = nc.dram_tensor("selfw", [128, cfg.NTILES], dt.float32,
                           kind="ExternalInput")
    SA, SB = plan.totA * 128, plan.totB * 128
    idxa = nc.dram_tensor("idxa", [128, SA // 16], dt.int16, kind="ExternalInput")
    idxb = nc.dram_tensor("idxb", [128, SB // 16], dt.int16, kind="ExternalInput")
    wsl = nc.dram_tensor("wsl", [128, plan.SLAB], dt.bfloat16, kind="ExternalInput")
    bias = nc.dram_tensor("bias", [128, F], dt.bfloat16, kind="ExternalInput")
    ident = nc.dram_tensor("ident", [128, 128], dt.bfloat16, kind="ExternalInput")
    if layer2:
        wnext = nc.dram_tensor("wnext", [128, (cfg.HID // 128) * cfg.OUT],
                               dt.bfloat16, kind="ExternalInput")
        out = nc.dram_tensor("out", [cfg.NP, cfg.OUT], dt.bfloat16,
                             kind="ExternalOutput")
    else:
        out = nc.dram_tensor("out", [cfg.NP, cfg.OUT], dt.float32,
                             kind="ExternalOutput")

    GC = cfg.GC
    nga = -(-plan.totA // GC) if plan.totA else 0
    ngb = -(-plan.totB // GC) if plan.totB else 0

    with tile.TileContext(nc) as tc, ExitStack() as ctx:
        consts = ctx.enter_context(tc.tile_pool(name="consts", bufs=1))
        gpa = ctx.enter_context(tc.tile_pool(name="gbufa", bufs=2))
        gpb = ctx.enter_context(tc.tile_pool(name="gbufb", bufs=2))
        work = ctx.enter_context(tc.tile_pool(name="work", bufs=3))
        psmp = ctx.enter_context(tc.tile_pool(name="psmp", bufs=2, space="PSUM"))
        pstr = ctx.enter_context(tc.tile_pool(name="pstr", bufs=2, space="PSUM"))
        psmm = ctx.enter_context(tc.tile_pool(name="psmm", bufs=2, space="PSUM"))

        def load_const(dram, shape, dtype, tag):
            t = consts.tile(shape, dtype, tag=tag)
            nc.sync.dma_start(t[:], dram[:])
            return t

        idxa_sb = load_const(idxa, [128, SA // 16], dt.int16, "idxa")
        idxb_sb = load_const(idxb, [128, SB // 16], dt.int16, "idxb")
        wsl_sb = load_const(wsl, [128, plan.SLAB], dt.bfloat16, "wsl")
        bias_sb = load_const(bias, [128, F], dt.bfloat16, "bias")
        ident_sb = load_const(ident, [128, 128], dt.bfloat16, "ident")
        selfw_sb = load_const(selfw, [128, cfg.NTILES], dt.float32, "selfw")
        if layer2:
            wnext_sb = load_const(wnext, [128, wnext.shape[1]], dt.bfloat16,
                                  "wnext")

        # gather groups, created lazily in consumption order
        gtiles = [{}, {}]

        def group_tile(h, g):
            if g in gtiles[h]:
                return gtiles[h][g]
            tot = plan.totA if h == 0 else plan.totB
            ck = min(GC, tot - g * GC)
            pool = gpa if h == 0 else gpb
            t = pool.tile([128, GC * F], dt.bfloat16)
            idx_sb = idxa_sb if h == 0 else idxb_sb
            half = tab[0:cfg.HALFROWS, :] if h == 0 else tab[cfg.HALFROWS:, :]
            nidx = ck * 128
            nc.gpsimd.dma_gather(
                out_ap=t[:, : ck * F].rearrange("p (c f) -> p c f", f=F),
                in_ap=half,
                idxs_ap=idx_sb[:, g * GC * 8: g * GC * 8 + ck * 8],
                num_idxs=nidx,
                num_idxs_reg=nidx,
                elem_size=F,
                queue_num=(h * 2 + g) % 4,
                single_packet=False,
            )
            gtiles[h][g] = t
            return t

        for t in range(cfg.NTILES):
            # chunk list for this tile in consumption order
            chunks = []
            for h, basearr in ((0, plan.abase), (1, plan.bbase)):
                for j in range(int(basearr[t]), int(basearr[t + 1])):
                    M = int(plan.baseM[h][1][j])
                    if M == 0:
                        continue
                    chunks.append((h, j, int(plan.baseM[h][0][j]), M,
                                   int(plan.slab_off[h][j])))

            # group bracketed by two half-bias matmuls so that start/stop
            # cover the full [0:128] region (sim zero-region discipline)
            ps = psmp.tile([128, F], dt.float32)
            nc.tensor.matmul(ps[:], ident_sb[:], bias_sb[:],
                             start=True, stop=False, skip_group_check=True)
            # dense self-loop term: scaled rows of this core's own shard
            ts_t = work.tile([128, F], dt.bfloat16, tag="ts")
            nc.sync.dma_start(ts_t[:], tabself[t * 128:(t + 1) * 128, :])
            sc_t = work.tile([128, F], dt.bfloat16, tag="sc")
            nc.scalar.activation(sc_t[:], ts_t[:],
                                 mybir.ActivationFunctionType.Copy,
                                 scale=selfw_sb[:, t:t + 1])
            nc.tensor.matmul(ps[:], ident_sb[:], sc_t[:],
                             start=False, stop=False, skip_group_check=True)
            for h, j, b0, M, so in chunks:
                gt = group_tile(h, j // GC)
                slot = j % GC
                rhs = gt[:, slot * F:(slot + 1) * F]
                for s in range(3):
                    Mq = int(plan.pieces[h][j, s])
                    if Mq == 0:
                        continue
                    bs = (0, 32, 64)[s]
                    col = so + bs - b0
                    nc.tensor.matmul(
                        ps[bs:bs + Mq, :],
                        wsl_sb[:, col:col + Mq],
                        rhs,
                        start=False, stop=False,
                        skip_group_check=True,
                    )
            nc.tensor.matmul(ps[:], ident_sb[:], bias_sb[:],
                             start=False, stop=True, skip_group_check=True)

            # post-processing
            if layer2:
                act = work.tile([128, F], dt.bfloat16)
                nc.scalar.activation(act[:], ps[:],
                                     mybir.ActivationFunctionType.Relu)
                trp = pstr.tile([128, F], dt.bfloat16)
                for c in range(FCH):
                    nc.tensor.transpose(trp[:, c * 128:(c + 1) * 128],
                                        act[:, c * 128:(c + 1) * 128],
                                        ident_sb[:])
                actT = work.tile([128, F], dt.bfloat16)
                nc.vector.tensor_copy(actT[:], trp[:])

                ps2 = psmm.tile([128, cfg.OUT], dt.float32)
                for c in range(FCH):
                    nc.tensor.matmul(ps2[:], actT[:, c * 128:(c + 1) * 128],
                                     wnext_sb[:, c * cfg.OUT:(c + 1) * cfg.OUT],
                                     start=(c == 0), stop=(c == FCH - 1))
                o = work.tile([128, cfg.OUT], dt.bfloat16)
                nc.scalar.activation(o[:], ps2[:],
                                     mybir.ActivationFunctionType.Copy)
            else:
                o = work.tile([128, cfg.OUT], dt.float32)
                nc.scalar.activation(o[:], ps[:],
                                     mybir.ActivationFunctionType.Copy)
            nc.sync.dma_start(out[t * 128:(t + 1) * 128, :], o[:])

    nc.finalize()
    return nc


# ---------------------------------------------------------------- host packing

def _pack_l1_inputs(cfg: Cfg, plan: Plan, x, W1):
    KCH = cfg.IN_DIM // 128
    w1r = np.zeros((128, KCH * cfg.HID), BF16)
    for c in range(KCH):
        w1r[:, c * cfg.HID:(c + 1) * cfg.HID] = W1[c * 128:(c + 1) * 128, :].astype(BF16)
    maps = []
    for k in range(cfg.NCORES):
        xs = np.zeros((cfg.NP, cfg.IN_DIM), np.float32)
        xs[:cfg.ND] = x[plan.nodes[k]]
        xtr = np.zeros((128, KCH * cfg.NP), BF16)
        for c in range(KCH):
            xtr[:, c * cfg.NP:(c + 1) * cfg.NP] = \
                xs[:, c * 128:(c + 1) * 128].T.astype(BF16)
        maps.append({"xt": xtr, "w1": w1r})
    return maps


def _pack_mp_inputs(cfg: Cfg, plan: Plan, table, Wn, b, layer2):
    F = cfg.HID if layer2 else cfg.OUT
    # the bias matmul runs twice per tile (group start + stop) -> send b/2
    biasr = np.tile((b * 0.5).astype(BF16)[None, :], (128, 1))
    ident = np.eye(128, dtype=BF16)
    maps = []
    for k in range(cfg.NCORES):
        ia, ib = plan.idxs[k]
        m = {
            "tab": table,
            "tabself": np.ascontiguousarray(
                table[k * cfg.NP:(k + 1) * cfg.NP]),
            "selfw": plan.selfw[k],
            "idxa": ia,
            "idxb": ib,
            "wsl": plan.wslab[k],
            "bias": biasr,
            "ident": ident,
        }
        if layer2:
            FCH = cfg.HID // 128
            wnr = np.zeros((128, FCH * cfg.OUT), BF16)
            for c in range(FCH):
                wnr[:, c * cfg.OUT:(c + 1) * cfg.OUT] = \
                    Wn[c * 128:(c + 1) * 128, :].astype(BF16)
            m["wnext"] = wnr
        maps.append(m)
    return maps


# ---------------------------------------------------------------- driver

def _run(nc, in_maps, cfg, trace=False):
    from concourse.bass_utils import run_bass_kernel_spmd
    res = run_bass_kernel_spmd(nc, in_maps, list(range(cfg.NCORES)), trace=trace)
    return res


def kernel_run(inputs, cfg=None, trace=False, sim=False):
    cfg = cfg or Cfg()
    x = np.asarray(inputs["x"], np.float32)
    plan = Plan(cfg, np.asarray(inputs["edge_index"]),
                np.asarray(inputs["edge_weight"], np.float32))
    W1 = np.asarray(inputs["W1"], np.float32)
    b1 = np.asarray(inputs["b1"], np.float32)
    W2 = np.asarray(inputs["W2"], np.float32)
    b2 = np.asarray(inputs["b2"], np.float32)
    Wp = np.asarray(inputs["Wp"], np.float32)
    bp = np.asarray(inputs["bp"], np.float32)

    results = []

    def run(build, maps, outname):
        nc = build()
        if sim:
            from concourse.bass_interp import CoreSim
            outs = []
            for k in range(cfg.NCORES):
                s = CoreSim(nc)
                for name, arr in maps[k].items():
                    s.tensor(name)[:] = arr
                s.simulate()
                outs.append({outname: s.tensor(outname).copy()})
            results.append(None)
            return outs
        r = _run(nc, maps, cfg, trace=trace)
        results.append(r)
        return r.results

    # fold the post-projection into layer 2: A(relu1@W2)@Wp = A(relu1@(W2@Wp))
    W2p = (W2 @ Wp).astype(np.float32)
    bpp = (b2 @ Wp + bp).astype(np.float32)

    r1 = run(lambda: _build_l1(cfg), _pack_l1_inputs(cfg, plan, x, W1), "h1")
    T1 = np.concatenate([np.asarray(r["h1"]).view(BF16) if r["h1"].dtype != BF16
                         else r["h1"] for r in r1], axis=0)

    r2 = run(lambda: _build_mp(cfg, plan, True),
             _pack_mp_inputs(cfg, plan, T1, W2p, b1, True), "out")
    T2 = np.concatenate([np.asarray(r["out"]).view(BF16)
                         if r["out"].dtype != BF16 else r["out"]
                         for r in r2], axis=0)

    r3 = run(lambda: _build_mp(cfg, plan, False),
             _pack_mp_inputs(cfg, plan, T2, None, bpp, False), "out")

    y = np.empty((cfg.N, cfg.OUT), np.float32)
    for k in range(cfg.NCORES):
        shard = np.asarray(r3[k]["out"], np.float32)
        y[plan.nodes[k]] = shard[:cfg.ND]
    return y, results


def kernel(**inputs):
    y, _ = kernel_run(inputs)
    return y



# revision 11
# speedup vs baseline: 1.3230x; 1.3230x over previous
"""Trainium2 Bass kernel: 2-layer GCN (GCNConv -> ReLU -> GCNConv -> Linear).

Strategy (8 NeuronCores, SPMD):
  - Destination-node sharding: core k owns nodes [k*6250, (k+1)*6250).
  - 3 launches with host-side exchange of the (small) activation tables:
      L1: H1 = X @ W1            (row-sharded dense matmul)
      L2: MP1 + bias + ReLU, then @ W2 -> H2   (message passing via dma_gather
          + PE segment-reduction with host-built one-hot*norm weight blocks)
      L3: MP2 + bias, then @ Wp + bp -> out
  - Message passing: edges sorted by destination; gathered source rows land on
    partitions (edge position mod 128); a [128, M] one-hot-times-norm block
    matrix (lhsT) contracts 128 edges into the destination rows of a PSUM tile.
    PSUM accumulates across chunks; a bias matmul (identity x replicated-bias)
    initializes every row first.
  - int16 gather indices => table split in two halves (cores 0-3 / 4-7).
  - All matmul operands bf16 (fp32 PSUM accumulation); final output fp32.
"""

import os
from contextlib import ExitStack
from dataclasses import dataclass, field

import numpy as np
import ml_dtypes

BF16 = ml_dtypes.bfloat16
FP32 = np.float32


# ---------------------------------------------------------------- config

@dataclass
class Cfg:
    N: int = 50000
    IN_DIM: int = 512
    HID: int = 256
    OUT: int = 128
    NCORES: int = 8
    GC: int = 32          # chunks per gather (4096 idxs; needs single_packet=False)

    ND: int = field(init=False)
    NTILES: int = field(init=False)
    NP: int = field(init=False)
    TROWS: int = field(init=False)
    HALFROWS: int = field(init=False)
    SRC_SPLIT: int = field(init=False)

    def __post_init__(self):
        self.ND = self.N // self.NCORES
        self.NTILES = (self.ND + 127) // 128
        self.NP = self.NTILES * 128
        self.TROWS = self.NCORES * self.NP
        self.HALFROWS = self.TROWS // 2
        self.SRC_SPLIT = (self.NCORES // 2) * self.ND
        assert self.HALFROWS <= 32768, "int16 gather index limit"


# ---------------------------------------------------------------- planner

class Plan:
    """Static (cross-core identical) geometry + per-core data arrays."""

    def __init__(self, cfg: Cfg, edge_index, edge_weight):
        self.cfg = cfg
        N, ND, NP, NT = cfg.N, cfg.ND, cfg.NP, cfg.NTILES
        NC = cfg.NCORES

        # --- gcn_norm with self loops (host: O(E) index/weight preprocessing)
        row = np.concatenate([np.asarray(edge_index[0], np.int64),
                              np.arange(N, dtype=np.int64)])
        col = np.concatenate([np.asarray(edge_index[1], np.int64),
                              np.arange(N, dtype=np.int64)])
        w = np.concatenate([np.asarray(edge_weight, np.float64),
                            np.ones(N, np.float64)])
        deg = np.zeros(N, np.float64)
        np.add.at(deg, col, w)
        dinv = np.where(deg > 0, 1.0 / np.sqrt(deg), 0.0)
        nrm = (dinv[row] * w * dinv[col]).astype(np.float32)

        # --- global degree-sorted serpentine node->(core, lane) assignment:
        # every core gets a near-identical degree profile, so the cross-core
        # max padding of the static chunk geometry nearly vanishes.
        degi = np.bincount(col, minlength=N)
        ranks = np.argsort(-degi, kind="stable")    # rank r -> node
        r = np.arange(N)
        blk = r // NC
        corepos = np.where(blk % 2 == 0, r % NC, NC - 1 - (r % NC))
        lane_r = blk
        lane_global = np.empty(N, np.int64)        # node -> core*NP + lane
        lane_global[ranks] = corepos * NP + lane_r
        self.nodes = []                             # per core: lane -> node id
        for k in range(NC):
            nk = np.empty(ND, np.int64)
            sel = corepos == k
            nk[lane_r[sel]] = ranks[sel]
            self.nodes.append(nk)

        # self loops handled densely (tables are assignment-ordered); their
        # weight is dinv^2 * 1.0
        self.selfw = []
        for k in range(NC):
            sw = np.zeros((128, NT), np.float32)
            lanes = np.arange(ND)
            vals = (dinv[self.nodes[k]] ** 2).astype(np.float32)
            sw[lanes % 128, lanes // 128] = vals
            self.selfw.append(sw)

        # drop only the APPENDED self-loop block (original (u,u) edges stay)
        ne = len(row) - N
        row, col, nrm = row[:ne], col[:ne], nrm[:ne]

        trow2 = lane_global[row]                    # table row of the source
        half = (trow2 >= cfg.HALFROWS).astype(np.int64)
        idx2 = np.where(half == 0, trow2, trow2 - cfg.HALFROWS)
        assert idx2.min() >= 0 and idx2.max() < cfg.HALFROWS

        dst_core = lane_global[col] // NP
        dlane = lane_global[col] % NP
        dtile = dlane // 128

        order = np.lexsort((dlane, half, dtile, dst_core))
        so_core = dst_core[order]
        so_tile = dtile[order]
        so_half = half[order]
        so_lane = (dlane - dtile * 128)[order]
        so_i2 = idx2[order]
        so_w = nrm[order]

        # edges per (core, tile, half)
        key = (so_core * NT + so_tile) * 2 + so_half
        cnt = np.bincount(key, minlength=NC * NT * 2).reshape(NC, NT, 2)
        Cch = -(-cnt // 128)                         # ceil chunks per seg
        self.CH = Cch.max(axis=0)                    # [NT, 2] static
        # stream chunk bases per (tile, half)
        self.abase = np.concatenate([[0], np.cumsum(self.CH[:, 0])])  # [NT+1]
        self.bbase = np.concatenate([[0], np.cumsum(self.CH[:, 1])])
        self.totA = int(self.abase[-1])
        self.totB = int(self.bbase[-1])
        SA, SB = self.totA * 128, self.totB * 128

        # edge position within its padded stream
        # rank within segment:
        seg_start_sorted = np.concatenate([[0], np.cumsum(np.bincount(
            key, minlength=NC * NT * 2))])[:-1]
        rank = np.arange(len(key)) - seg_start_sorted[key]
        base_chunks = np.where(so_half == 0,
                               self.abase[so_tile],
                               self.bbase[so_tile])
        pos = base_chunks * 128 + rank               # position in its stream
        chunk = base_chunks + rank // 128            # stream chunk index
        lanepos = pos % 128

        # --- chunk windows (cross-core): base lane / M per (half, chunk)
        self.baseM = []
        for h, tot in ((0, self.totA), (1, self.totB)):
            m = so_half == h
            mn = np.full(tot, 128, np.int64)
            mx = np.full(tot, -1, np.int64)
            np.minimum.at(mn, chunk[m], so_lane[m])
            np.maximum.at(mx, chunk[m], so_lane[m])
            empty = mx < 0
            mn[empty] = 0
            # Legal matmul out windows: base 0 (M<=128), base 32 (M<=32),
            # base 64 (M<=64).  Slab window starts at min(32*(mn//32), 64).
            mn = np.minimum((mn // 32) * 32, 64)
            M = np.where(empty, 0, mx - mn + 1)
            self.baseM.append((mn, M))

        # matmul pieces per chunk: slots with bases (0, 32, 64); lanes >= 64
        # all go to the base-64 slot (M<=64 there, legal)
        self.pieces = []
        for h, tot in ((0, self.totA), (1, self.totB)):
            m = so_half == h
            slot = np.minimum(so_lane[m] // 32, 2)
            key2 = chunk[m] * 3 + slot
            mx2 = np.full(max(tot, 1) * 3, -1, np.int64)
            np.maximum.at(mx2, key2, so_lane[m])
            mx2 = mx2.reshape(-1, 3)[:tot]
            Ms = np.where(mx2 >= 0, mx2 - np.array([0, 32, 64]) + 1, 0)
            self.pieces.append(Ms)

        # consumption order (tile: A chunks then B chunks) -> slab offsets
        self.slab_off = [np.zeros(self.totA, np.int64),
                         np.zeros(self.totB, np.int64)]
        off = 0
        for t in range(NT):
            for h, base in ((0, self.abase), (1, self.bbase)):
                for j in range(int(base[t]), int(base[t + 1])):
                    self.slab_off[h][j] = off
                    off += int(self.baseM[h][1][j])
        self.SLAB = max(off, 1)

        # --- per-core arrays
        self.idxs = []   # (idxA, idxB) wrapped int16 [128, S/16]
        self.wslab = []  # [128, SLAB] bf16
        for k in range(NC):
            m = so_core == k
            kh, kpos, kchunk, klp = so_half[m], pos[m], chunk[m], lanepos[m]
            ki2, kw, klane = so_i2[m], so_w[m], so_lane[m]

            arrs = []
            for h, S in ((0, SA), (1, SB)):
                hm = kh == h
                lin = np.zeros(S, np.int16)
                lin[kpos[hm]] = ki2[hm].astype(np.int16)
                arrs.append(self._wrap16(lin))
            self.idxs.append((arrs[0], arrs[1]))

            slab = np.zeros((128, self.SLAB), np.float32)
            colw = self.slab_off[0] - self.baseM[0][0]
            colwB = self.slab_off[1] - self.baseM[1][0]
            hm = kh == 0
            slab[klp[hm], kchunk[hm] * 0 + colw[kchunk[hm]] + klane[hm]] = kw[hm]
            hm = kh == 1
            slab[klp[hm], colwB[kchunk[hm]] + klane[hm]] = kw[hm]
            self.wslab.append(slab.astype(BF16))

    @staticmethod
    def _wrap16(lin):
        # position i lives at [i % 16, i // 16]; replicated to 128 partitions
        w = lin.reshape(-1, 16).T.copy()
        return np.tile(w, (8, 1))


# ---------------------------------------------------------------- bass builders

def _build_l1(cfg: Cfg):
    import concourse.bacc as bacc
    import concourse.mybir as mybir
    import concourse.tile as tile

    dt = mybir.dt
    nc = bacc.Bacc(None, target_bir_lowering=False, num_swdge_queues=4)
    KCH = cfg.IN_DIM // 128
    xt = nc.dram_tensor("xt", [128, KCH * cfg.NP], dt.bfloat16, kind="ExternalInput")
    w1 = nc.dram_tensor("w1", [128, KCH * cfg.HID], dt.bfloat16, kind="ExternalInput")
    h1 = nc.dram_tensor("h1", [cfg.NP, cfg.HID], dt.bfloat16, kind="ExternalOutput")

    XG = 10                       # tiles per xt load chunk (overlap DMA/PE)
    NG = -(-cfg.NTILES // XG)
    with tile.TileContext(nc) as tc, ExitStack() as ctx:
        consts = ctx.enter_context(tc.tile_pool(name="consts", bufs=1))
        outs = ctx.enter_context(tc.tile_pool(name="outs", bufs=3))
        psum = ctx.enter_context(tc.tile_pool(name="psum", bufs=2, space="PSUM"))

        w1_sb = consts.tile([128, KCH * cfg.HID], dt.bfloat16, tag="w1")
        nc.sync.dma_start(w1_sb[:], w1[:])
        xts = []
        for g in range(NG):
            tg = min(XG, cfg.NTILES - g * XG)
            xg = consts.tile([128, KCH * tg * 128], dt.bfloat16, tag=f"xt{g}")
            for c in range(KCH):
                nc.sync.dma_start(
                    xg[:, c * tg * 128:(c + 1) * tg * 128],
                    xt[:, c * cfg.NP + g * XG * 128:
                       c * cfg.NP + (g * XG + tg) * 128])
            xts.append((xg, tg))

        for t in range(cfg.NTILES):
            g, j = t // XG, t % XG
            xg, tg = xts[g]
            ps = psum.tile([128, cfg.HID], dt.float32)
            for c in range(KCH):
                nc.tensor.matmul(
                    ps[:],
                    xg[:, c * tg * 128 + j * 128: c * tg * 128 + (j + 1) * 128],
                    w1_sb[:, c * cfg.HID:(c + 1) * cfg.HID],
                    start=(c == 0), stop=(c == KCH - 1),
                )
            o = outs.tile([128, cfg.HID], dt.bfloat16)
            nc.scalar.activation(o[:], ps[:], mybir.ActivationFunctionType.Copy)
            nc.sync.dma_start(h1[t * 128:(t + 1) * 128, :], o[:])
    nc.finalize()
    return nc


def _build_mp(cfg: Cfg, plan: Plan, layer2: bool):
    """layer2: MP1 + ReLU + @W2 -> H2 (bf16). else: MP2 + @Wp + bp -> y (f32)."""
    import concourse.bacc as bacc
    import concourse.mybir as mybir
    import concourse.tile as tile

    dt = mybir.dt
    F = cfg.HID if layer2 else cfg.OUT           # table feature width
    FCH = F // 128
    nc = bacc.Bacc(None, target_bir_lowering=False, num_swdge_queues=4)

    tab = nc.dram_tensor("tab", [cfg.TROWS, F], dt.bfloat16, kind="ExternalInput")
    tabself = nc.dram_tensor("tabself", [cfg.NP, F], dt.bfloat16,
                             kind="ExternalInput")
    selfw = nc.dram_tensor("selfw", [128, cfg.NTILES], dt.float32,
                           kind="ExternalInput")
    SA, SB = plan.totA * 128, plan.totB * 128
    idxa = nc.dram_tensor("idxa", [128, SA // 16], dt.int16, kind="ExternalInput")
    idxb = nc.dram_tensor("idxb", [128, SB // 16], dt.int16, kind="ExternalInput")
    wsl = nc.dram_tensor("wsl", [128, plan.SLAB], dt.bfloat16, kind="ExternalInput")
    bias = nc.dram_tensor("bias", [128, F], dt.bfloat16, kind="ExternalInput")
    ident = nc.dram_tensor("ident", [128, 128], dt.bfloat16, kind="ExternalInput")
    if layer2:
        wnext = nc.dram_tensor("wnext", [128, (cfg.HID // 128) * cfg.OUT],
                               dt.bfloat16, kind="ExternalInput")
        out = nc.dram_tensor("out", [cfg.NP, cfg.OUT], dt.bfloat16,
                             kind="ExternalOutput")
    else:
        out = nc.dram_tensor("out", [cfg.NP, cfg.OUT], dt.float32,
                             kind="ExternalOutput")

    GC = cfg.GC
    nga = -(-plan.totA // GC) if plan.totA else 0
    ngb = -(-plan.totB // GC) if plan.totB else 0

    with tile.TileContext(nc) as tc, ExitStack() as ctx:
        consts = ctx.enter_context(tc.tile_pool(name="consts", bufs=1))
        gpa = ctx.enter_context(tc.tile_pool(name="gbufa", bufs=3))
        gpb = ctx.enter_context(tc.tile_pool(name="gbufb", bufs=3))
        work = ctx.enter_context(tc.tile_pool(name="work", bufs=3))
        psmp = ctx.enter_context(tc.tile_pool(name="psmp", bufs=2, space="PSUM"))
        pstr = ctx.enter_context(tc.tile_pool(name="pstr", bufs=2, space="PSUM"))
        psmm = ctx.enter_context(tc.tile_pool(name="psmm", bufs=2, space="PSUM"))

        def load_const(dram, shape, dtype, tag):
            t = consts.tile(shape, dtype, tag=tag)
            nc.sync.dma_start(t[:], dram[:])
            return t

        idxa_sb = load_const(idxa, [128, SA // 16], dt.int16, "idxa")
        idxb_sb = load_const(idxb, [128, SB // 16], dt.int16, "idxb")
        wsl_sb = load_const(wsl, [128, plan.SLAB], dt.bfloat16, "wsl")
        bias_sb = load_const(bias, [128, F], dt.bfloat16, "bias")
        ident_sb = load_const(ident, [128, 128], dt.bfloat16, "ident")
        selfw_sb = load_const(selfw, [128, cfg.NTILES], dt.float32, "selfw")
        if layer2:
            wnext_sb = load_const(wnext, [128, wnext.shape[1]], dt.bfloat16,
                                  "wnext")

        # gather groups, created lazily in consumption order
        gtiles = [{}, {}]

        def group_tile(h, g):
            if g in gtiles[h]:
                return gtiles[h][g]
            tot = plan.totA if h == 0 else plan.totB
            ck = min(GC, tot - g * GC)
            pool = gpa if h == 0 else gpb
            t = pool.tile([128, GC * F], dt.bfloat16)
            idx_sb = idxa_sb if h == 0 else idxb_sb
            half = tab[0:cfg.HALFROWS, :] if h == 0 else tab[cfg.HALFROWS:, :]
            nidx = ck * 128
            nc.gpsimd.dma_gather(
                out_ap=t[:, : ck * F].rearrange("p (c f) -> p c f", f=F),
                in_ap=half,
                idxs_ap=idx_sb[:, g * GC * 8: g * GC * 8 + ck * 8],
                num_idxs=nidx,
                num_idxs_reg=nidx,
                elem_size=F,
                queue_num=(h * 2 + g) % 4,
                single_packet=False,
            )
            gtiles[h][g] = t
            return t

        for t in range(cfg.NTILES):
            # chunk list for this tile in consumption order
            chunks = []
            for h, basearr in ((0, plan.abase), (1, plan.bbase)):
                for j in range(int(basearr[t]), int(basearr[t + 1])):
                    M = int(plan.baseM[h][1][j])
                    if M == 0:
                        continue
                    chunks.append((h, j, int(plan.baseM[h][0][j]), M,
                                   int(plan.slab_off[h][j])))

            # group bracketed by two half-bias matmuls so that start/stop
            # cover the full [0:128] region (sim zero-region discipline)
            ps = psmp.tile([128, F], dt.float32)
            nc.tensor.matmul(ps[:], ident_sb[:], bias_sb[:],
                             start=True, stop=False, skip_group_check=True)
            # dense self-loop term: scaled rows of this core's own shard
            ts_t = work.tile([128, F], dt.bfloat16, tag="ts")
            nc.scalar.dma_start(ts_t[:], tabself[t * 128:(t + 1) * 128, :])
            sc_t = work.tile([128, F], dt.bfloat16, tag="sc")
            nc.scalar.activation(sc_t[:], ts_t[:],
                                 mybir.ActivationFunctionType.Copy,
                                 scale=selfw_sb[:, t:t + 1])
            nc.tensor.matmul(ps[:], ident_sb[:], sc_t[:],
                             start=False, stop=False, skip_group_check=True)
            for h, j, b0, M, so in chunks:
                gt = group_tile(h, j // GC)
                slot = j % GC
                rhs = gt[:, slot * F:(slot + 1) * F]
                for s in range(3):
                    Mq = int(plan.pieces[h][j, s])
                    if Mq == 0:
                        continue
                    bs = (0, 32, 64)[s]
                    col = so + bs - b0
                    nc.tensor.matmul(
                        ps[bs:bs + Mq, :],
                        wsl_sb[:, col:col + Mq],
                        rhs,
                        start=False, stop=False,
                        skip_group_check=True,
                    )
            nc.tensor.matmul(ps[:], ident_sb[:], bias_sb[:],
                             start=False, stop=True, skip_group_check=True)

            # post-processing
            if layer2:
                act = work.tile([128, F], dt.bfloat16)
                nc.scalar.activation(act[:], ps[:],
                                     mybir.ActivationFunctionType.Relu)
                trp = pstr.tile([128, F], dt.bfloat16)
                for c in range(FCH):
                    nc.tensor.transpose(trp[:, c * 128:(c + 1) * 128],
                                        act[:, c * 128:(c + 1) * 128],
                                        ident_sb[:])
                actT = work.tile([128, F], dt.bfloat16)
                nc.vector.tensor_copy(actT[:], trp[:])

                ps2 = psmm.tile([128, cfg.OUT], dt.float32)
                for c in range(FCH):
                    nc.tensor.matmul(ps2[:], actT[:, c * 128:(c + 1) * 128],
                                     wnext_sb[:, c * cfg.OUT:(c + 1) * cfg.OUT],
                                     start=(c == 0), stop=(c == FCH - 1))
                o = work.tile([128, cfg.OUT], dt.bfloat16)
                nc.scalar.activation(o[:], ps2[:],
                                     mybir.ActivationFunctionType.Copy)
            else:
                o = work.tile([128, cfg.OUT], dt.float32)
                nc.scalar.activation(o[:], ps[:],
                                     mybir.ActivationFunctionType.Copy)
            nc.scalar.dma_start(out[t * 128:(t + 1) * 128, :], o[:])

    nc.finalize()
    return nc


# ---------------------------------------------------------------- host packing

def _pack_l1_inputs(cfg: Cfg, plan: Plan, x, W1):
    KCH = cfg.IN_DIM // 128
    w1r = np.zeros((128, KCH * cfg.HID), BF16)
    for c in range(KCH):
        w1r[:, c * cfg.HID:(c + 1) * cfg.HID] = W1[c * 128:(c + 1) * 128, :].astype(BF16)
    maps = []
    for k in range(cfg.NCORES):
        xs = np.zeros((cfg.NP, cfg.IN_DIM), np.float32)
        xs[:cfg.ND] = x[plan.nodes[k]]
        xtr = np.zeros((128, KCH * cfg.NP), BF16)
        for c in range(KCH):
            xtr[:, c * cfg.NP:(c + 1) * cfg.NP] = \
                xs[:, c * 128:(c + 1) * 128].T.astype(BF16)
        maps.append({"xt": xtr, "w1": w1r})
    return maps


def _pack_mp_inputs(cfg: Cfg, plan: Plan, table, Wn, b, layer2):
    F = cfg.HID if layer2 else cfg.OUT
    # the bias matmul runs twice per tile (group start + stop) -> send b/2
    biasr = np.tile((b * 0.5).astype(BF16)[None, :], (128, 1))
    ident = np.eye(128, dtype=BF16)
    maps = []
    for k in range(cfg.NCORES):
        ia, ib = plan.idxs[k]
        m = {
            "tab": table,
            "tabself": np.ascontiguousarray(
                table[k * cfg.NP:(k + 1) * cfg.NP]),
            "selfw": plan.selfw[k],
            "idxa": ia,
            "idxb": ib,
            "wsl": plan.wslab[k],
            "bias": biasr,
            "ident": ident,
        }
        if layer2:
            FCH = cfg.HID // 128
            wnr = np.zeros((128, FCH * cfg.OUT), BF16)
            for c in range(FCH):
                wnr[:, c * cfg.OUT:(c + 1) * cfg.OUT] = \
                    Wn[c * 128:(c + 1) * 128, :].astype(BF16)
            m["wnext"] = wnr
        maps.append(m)
    return maps


# ---------------------------------------------------------------- driver

def _run(nc, in_maps, cfg, trace=False):
    from concourse.bass_utils import run_bass_kernel_spmd
    res = run_bass_kernel_spmd(nc, in_maps, list(range(cfg.NCORES)), trace=trace)
    return res


def kernel_run(inputs, cfg=None, trace=False, sim=False):
    cfg = cfg or Cfg()
    x = np.asarray(inputs["x"], np.float32)
    plan = Plan(cfg, np.asarray(inputs["edge_index"]),
                np.asarray(inputs["edge_weight"], np.float32))
    W1 = np.asarray(inputs["W1"], np.float32)
    b1 = np.asarray(inputs["b1"], np.float32)
    W2 = np.asarray(inputs["W2"], np.float32)
    b2 = np.asarray(inputs["b2"], np.float32)
    Wp = np.asarray(inputs["Wp"], np.float32)
    bp = np.asarray(inputs["bp"], np.float32)

    results = []

    def run(build, maps, outname):
        nc = build()
        if sim:
            from concourse.bass_interp import CoreSim
            outs = []
            for k in range(cfg.NCORES):
                s = CoreSim(nc)
                for name, arr in maps[k].items():
                    s.tensor(name)[:] = arr
                s.simulate()
                outs.append({outname: s.tensor(outname).copy()})
            results.append(None)
            return outs
        r = _run(nc, maps, cfg, trace=trace)
        results.append(r)
        return r.results

    # fold the post-projection into layer 2: A(relu1@W2)@Wp = A(relu1@(W2@Wp))
    W2p = (W2 @ Wp).astype(np.float32)
    bpp = (b2 @ Wp + bp).astype(np.float32)

    r1 = run(lambda: _build_l1(cfg), _pack_l1_inputs(cfg, plan, x, W1), "h1")
    T1 = np.concatenate([np.asarray(r["h1"]).view(BF16) if r["h1"].dtype != BF16
                         else r["h1"] for r in r1], axis=0)

    r2 = run(lambda: _build_mp(cfg, plan, True),
             _pack_mp_inputs(cfg, plan, T1, W2p, b1, True), "out")
    T2 = np.concatenate([np.asarray(r["out"]).view(BF16)
                         if r["out"].dtype != BF16 else r["out"]
                         for r in r2], axis=0)

    r3 = run(lambda: _build_mp(cfg, plan, False),
             _pack_mp_inputs(cfg, plan, T2, None, bpp, False), "out")

    y = np.empty((cfg.N, cfg.OUT), np.float32)
    for k in range(cfg.NCORES):
        shard = np.asarray(r3[k]["out"], np.float32)
        y[plan.nodes[k]] = shard[:cfg.ND]
    return y, results


def kernel(**inputs):
    y, _ = kernel_run(inputs)
    return y



# revision 20
# speedup vs baseline: 1.3352x; 1.0092x over previous
"""Trainium2 Bass kernel: 2-layer GCN (GCNConv -> ReLU -> GCNConv -> Linear).

Strategy (8 NeuronCores, SPMD):
  - Destination-node sharding: core k owns nodes [k*6250, (k+1)*6250).
  - 3 launches with host-side exchange of the (small) activation tables:
      L1: H1 = X @ W1            (row-sharded dense matmul)
      L2: MP1 + bias + ReLU, then @ W2 -> H2   (message passing via dma_gather
          + PE segment-reduction with host-built one-hot*norm weight blocks)
      L3: MP2 + bias, then @ Wp + bp -> out
  - Message passing: edges sorted by destination; gathered source rows land on
    partitions (edge position mod 128); a [128, M] one-hot-times-norm block
    matrix (lhsT) contracts 128 edges into the destination rows of a PSUM tile.
    PSUM accumulates across chunks; a bias matmul (identity x replicated-bias)
    initializes every row first.
  - int16 gather indices => table split in two halves (cores 0-3 / 4-7).
  - All matmul operands bf16 (fp32 PSUM accumulation); final output fp32.
"""

import os
from contextlib import ExitStack
from dataclasses import dataclass, field

import numpy as np
import ml_dtypes

BF16 = ml_dtypes.bfloat16
FP32 = np.float32


# ---------------------------------------------------------------- config

@dataclass
class Cfg:
    N: int = 50000
    IN_DIM: int = 512
    HID: int = 256
    OUT: int = 128
    NCORES: int = 8
    GC: int = 32          # chunks per gather (4096 idxs; needs single_packet=False)

    ND: int = field(init=False)
    NTILES: int = field(init=False)
    NP: int = field(init=False)
    TROWS: int = field(init=False)
    HALFROWS: int = field(init=False)
    SRC_SPLIT: int = field(init=False)

    def __post_init__(self):
        self.ND = self.N // self.NCORES
        self.NTILES = (self.ND + 127) // 128
        self.NP = self.NTILES * 128
        self.TROWS = self.NCORES * self.NP
        self.HALFROWS = self.TROWS // 2
        self.SRC_SPLIT = (self.NCORES // 2) * self.ND
        assert self.HALFROWS <= 32768, "int16 gather index limit"


# ---------------------------------------------------------------- planner

class Plan:
    """Static (cross-core identical) geometry + per-core data arrays."""

    def __init__(self, cfg: Cfg, edge_index, edge_weight):
        self.cfg = cfg
        N, ND, NP, NT = cfg.N, cfg.ND, cfg.NP, cfg.NTILES
        NC = cfg.NCORES

        # --- gcn_norm with self loops (host: O(E) index/weight preprocessing)
        row = np.concatenate([np.asarray(edge_index[0], np.int64),
                              np.arange(N, dtype=np.int64)])
        col = np.concatenate([np.asarray(edge_index[1], np.int64),
                              np.arange(N, dtype=np.int64)])
        w = np.concatenate([np.asarray(edge_weight, np.float64),
                            np.ones(N, np.float64)])
        deg = np.zeros(N, np.float64)
        np.add.at(deg, col, w)
        dinv = np.where(deg > 0, 1.0 / np.sqrt(deg), 0.0)
        nrm = (dinv[row] * w * dinv[col]).astype(np.float32)

        # --- global degree-sorted serpentine node->(core, lane) assignment:
        # every core gets a near-identical degree profile, so the cross-core
        # max padding of the static chunk geometry nearly vanishes.
        degi = np.bincount(col, minlength=N)
        ranks = np.argsort(-degi, kind="stable")    # rank r -> node
        r = np.arange(N)
        blk = r // NC
        corepos = np.where(blk % 2 == 0, r % NC, NC - 1 - (r % NC))
        lane_r = blk
        lane_global = np.empty(N, np.int64)        # node -> core*NP + lane
        lane_global[ranks] = corepos * NP + lane_r
        self.nodes = []                             # per core: lane -> node id
        for k in range(NC):
            nk = np.empty(ND, np.int64)
            sel = corepos == k
            nk[lane_r[sel]] = ranks[sel]
            self.nodes.append(nk)

        # self loops handled densely (tables are assignment-ordered); their
        # weight is dinv^2 * 1.0
        self.selfw = []
        for k in range(NC):
            sw = np.zeros((128, NT), np.float32)
            lanes = np.arange(ND)
            vals = (dinv[self.nodes[k]] ** 2).astype(np.float32)
            sw[lanes % 128, lanes // 128] = vals
            self.selfw.append(sw)

        # drop only the APPENDED self-loop block (original (u,u) edges stay)
        ne = len(row) - N
        row, col, nrm = row[:ne], col[:ne], nrm[:ne]

        trow2 = lane_global[row]                    # table row of the source
        half = (trow2 >= cfg.HALFROWS).astype(np.int64)
        idx2 = np.where(half == 0, trow2, trow2 - cfg.HALFROWS)
        assert idx2.min() >= 0 and idx2.max() < cfg.HALFROWS

        dst_core = lane_global[col] // NP
        dlane = lane_global[col] % NP
        dtile = dlane // 128

        order = np.lexsort((dlane, half, dtile, dst_core))
        so_core = dst_core[order]
        so_tile = dtile[order]
        so_half = half[order]
        so_lane = (dlane - dtile * 128)[order]
        so_i2 = idx2[order]
        so_w = nrm[order]

        # edges per (core, tile, half)
        key = (so_core * NT + so_tile) * 2 + so_half
        cnt = np.bincount(key, minlength=NC * NT * 2).reshape(NC, NT, 2)
        Cch = -(-cnt // 128)                         # ceil chunks per seg
        self.CH = Cch.max(axis=0)                    # [NT, 2] static
        # stream chunk bases per (tile, half)
        self.abase = np.concatenate([[0], np.cumsum(self.CH[:, 0])])  # [NT+1]
        self.bbase = np.concatenate([[0], np.cumsum(self.CH[:, 1])])
        self.totA = int(self.abase[-1])
        self.totB = int(self.bbase[-1])
        SA, SB = self.totA * 128, self.totB * 128

        # edge position within its padded stream
        # rank within segment:
        seg_start_sorted = np.concatenate([[0], np.cumsum(np.bincount(
            key, minlength=NC * NT * 2))])[:-1]
        rank = np.arange(len(key)) - seg_start_sorted[key]
        base_chunks = np.where(so_half == 0,
                               self.abase[so_tile],
                               self.bbase[so_tile])
        pos = base_chunks * 128 + rank               # position in its stream
        chunk = base_chunks + rank // 128            # stream chunk index
        lanepos = pos % 128

        # --- chunk windows (cross-core): base lane / M per (half, chunk)
        self.baseM = []
        for h, tot in ((0, self.totA), (1, self.totB)):
            m = so_half == h
            mn = np.full(tot, 128, np.int64)
            mx = np.full(tot, -1, np.int64)
            np.minimum.at(mn, chunk[m], so_lane[m])
            np.maximum.at(mx, chunk[m], so_lane[m])
            empty = mx < 0
            mn[empty] = 0
            # Legal matmul out windows: base 0 (M<=128), base 32 (M<=32),
            # base 64 (M<=64).  Slab window starts at min(32*(mn//32), 64).
            mn = np.minimum((mn // 32) * 32, 64)
            M = np.where(empty, 0, mx - mn + 1)
            self.baseM.append((mn, M))

        # matmul pieces per chunk: slots with bases (0, 32, 64); lanes >= 64
        # all go to the base-64 slot (M<=64 there, legal)
        self.pieces = []
        for h, tot in ((0, self.totA), (1, self.totB)):
            m = so_half == h
            slot = np.minimum(so_lane[m] // 32, 2)
            key2 = chunk[m] * 3 + slot
            mx2 = np.full(max(tot, 1) * 3, -1, np.int64)
            np.maximum.at(mx2, key2, so_lane[m])
            mx2 = mx2.reshape(-1, 3)[:tot]
            Ms = np.where(mx2 >= 0, mx2 - np.array([0, 32, 64]) + 1, 0)
            self.pieces.append(Ms)

        # consumption order (tile: A chunks then B chunks) -> slab offsets
        self.slab_off = [np.zeros(self.totA, np.int64),
                         np.zeros(self.totB, np.int64)]
        off = 0
        for t in range(NT):
            for h, base in ((0, self.abase), (1, self.bbase)):
                for j in range(int(base[t]), int(base[t + 1])):
                    self.slab_off[h][j] = off
                    off += int(self.baseM[h][1][j])
        self.SLAB = max(off, 1)

        # --- per-core arrays
        self.idxs = []   # (idxA, idxB) wrapped int16 [128, S/16]
        self.wslab = []  # [128, SLAB] bf16
        for k in range(NC):
            m = so_core == k
            kh, kpos, kchunk, klp = so_half[m], pos[m], chunk[m], lanepos[m]
            ki2, kw, klane = so_i2[m], so_w[m], so_lane[m]

            arrs = []
            for h, S in ((0, SA), (1, SB)):
                hm = kh == h
                lin = np.zeros(S, np.int16)
                lin[kpos[hm]] = ki2[hm].astype(np.int16)
                arrs.append(self._wrap16(lin))
            self.idxs.append((arrs[0], arrs[1]))

            slab = np.zeros((128, self.SLAB), np.float32)
            colw = self.slab_off[0] - self.baseM[0][0]
            colwB = self.slab_off[1] - self.baseM[1][0]
            hm = kh == 0
            slab[klp[hm], kchunk[hm] * 0 + colw[kchunk[hm]] + klane[hm]] = kw[hm]
            hm = kh == 1
            slab[klp[hm], colwB[kchunk[hm]] + klane[hm]] = kw[hm]
            self.wslab.append(slab.astype(BF16))

    @staticmethod
    def _wrap16(lin):
        # position i lives at [i % 16, i // 16]; replicated to 128 partitions
        w = lin.reshape(-1, 16).T.copy()
        return np.tile(w, (8, 1))


# ---------------------------------------------------------------- bass builders

def _build_l1(cfg: Cfg):
    import concourse.bacc as bacc
    import concourse.mybir as mybir
    import concourse.tile as tile

    dt = mybir.dt
    nc = bacc.Bacc(None, target_bir_lowering=False, num_swdge_queues=4)
    KCH = cfg.IN_DIM // 128
    xt = nc.dram_tensor("xt", [128, KCH * cfg.NP], dt.bfloat16, kind="ExternalInput")
    w1 = nc.dram_tensor("w1", [128, KCH * cfg.HID], dt.bfloat16, kind="ExternalInput")
    h1 = nc.dram_tensor("h1", [cfg.NP, cfg.HID], dt.bfloat16, kind="ExternalOutput")

    XG = 10                       # tiles per xt load chunk (overlap DMA/PE)
    NG = -(-cfg.NTILES // XG)
    with tile.TileContext(nc) as tc, ExitStack() as ctx:
        consts = ctx.enter_context(tc.tile_pool(name="consts", bufs=1))
        outs = ctx.enter_context(tc.tile_pool(name="outs", bufs=3))
        psum = ctx.enter_context(tc.tile_pool(name="psum", bufs=2, space="PSUM"))

        w1_sb = consts.tile([128, KCH * cfg.HID], dt.bfloat16, tag="w1")
        nc.sync.dma_start(w1_sb[:], w1[:])
        xts = []
        for g in range(NG):
            tg = min(XG, cfg.NTILES - g * XG)
            xg = consts.tile([128, KCH * tg * 128], dt.bfloat16, tag=f"xt{g}")
            for c in range(KCH):
                nc.sync.dma_start(
                    xg[:, c * tg * 128:(c + 1) * tg * 128],
                    xt[:, c * cfg.NP + g * XG * 128:
                       c * cfg.NP + (g * XG + tg) * 128])
            xts.append((xg, tg))

        for t in range(cfg.NTILES):
            g, j = t // XG, t % XG
            xg, tg = xts[g]
            ps = psum.tile([128, cfg.HID], dt.float32)
            for c in range(KCH):
                nc.tensor.matmul(
                    ps[:],
                    xg[:, c * tg * 128 + j * 128: c * tg * 128 + (j + 1) * 128],
                    w1_sb[:, c * cfg.HID:(c + 1) * cfg.HID],
                    start=(c == 0), stop=(c == KCH - 1),
                )
            o = outs.tile([128, cfg.HID], dt.bfloat16)
            nc.scalar.activation(o[:], ps[:], mybir.ActivationFunctionType.Copy)
            nc.sync.dma_start(h1[t * 128:(t + 1) * 128, :], o[:])
    nc.finalize()
    return nc


def _build_mp(cfg: Cfg, plan: Plan, layer2: bool):
    """layer2: MP1 + ReLU + @W2 -> H2 (bf16). else: MP2 + @Wp + bp -> y (f32)."""
    import concourse.bacc as bacc
    import concourse.mybir as mybir
    import concourse.tile as tile

    dt = mybir.dt
    F = cfg.HID if layer2 else cfg.OUT           # table feature width
    FCH = F // 128
    nc = bacc.Bacc(None, target_bir_lowering=False, num_swdge_queues=4)

    tab = nc.dram_tensor("tab", [cfg.TROWS, F], dt.bfloat16, kind="ExternalInput")
    tabself = nc.dram_tensor("tabself", [cfg.NP, F], dt.bfloat16,
                             kind="ExternalInput")
    selfw = nc.dram_tensor("selfw", [128, cfg.NTILES], dt.float32,
                           kind="ExternalInput")
    SA, SB = plan.totA * 128, plan.totB * 128
    idxa = nc.dram_tensor("idxa", [128, SA // 16], dt.int16, kind="ExternalInput")
    idxb = nc.dram_tensor("idxb", [128, SB // 16], dt.int16, kind="ExternalInput")
    wsl = nc.dram_tensor("wsl", [128, plan.SLAB], dt.bfloat16, kind="ExternalInput")
    bias = nc.dram_tensor("bias", [128, F], dt.bfloat16, kind="ExternalInput")
    ident = nc.dram_tensor("ident", [128, 128], dt.bfloat16, kind="ExternalInput")
    if layer2:
        wnext = nc.dram_tensor("wnext", [128, (cfg.HID // 128) * cfg.OUT],
                               dt.bfloat16, kind="ExternalInput")
        out = nc.dram_tensor("out", [cfg.NP, cfg.OUT], dt.bfloat16,
                             kind="ExternalOutput")
    else:
        out = nc.dram_tensor("out", [cfg.NP, cfg.OUT], dt.float32,
                             kind="ExternalOutput")

    GC = cfg.GC
    nga = -(-plan.totA // GC) if plan.totA else 0
    ngb = -(-plan.totB // GC) if plan.totB else 0

    with tile.TileContext(nc) as tc, ExitStack() as ctx:
        consts = ctx.enter_context(tc.tile_pool(name="consts", bufs=1))
        gpa = ctx.enter_context(tc.tile_pool(name="gbufa", bufs=3))
        gpb = ctx.enter_context(tc.tile_pool(name="gbufb", bufs=3))
        work = ctx.enter_context(tc.tile_pool(name="work", bufs=3))
        psmp = ctx.enter_context(tc.tile_pool(name="psmp", bufs=2, space="PSUM"))
        pstr = ctx.enter_context(tc.tile_pool(name="pstr", bufs=2, space="PSUM"))
        psmm = ctx.enter_context(tc.tile_pool(name="psmm", bufs=2, space="PSUM"))

        def load_const(dram, shape, dtype, tag):
            t = consts.tile(shape, dtype, tag=tag)
            nc.sync.dma_start(t[:], dram[:])
            return t

        idxa_sb = load_const(idxa, [128, SA // 16], dt.int16, "idxa")
        idxb_sb = load_const(idxb, [128, SB // 16], dt.int16, "idxb")
        wsl_sb = load_const(wsl, [128, plan.SLAB], dt.bfloat16, "wsl")
        bias_sb = load_const(bias, [128, F], dt.bfloat16, "bias")
        ident_sb = load_const(ident, [128, 128], dt.bfloat16, "ident")
        selfw_sb = load_const(selfw, [128, cfg.NTILES], dt.float32, "selfw")
        if layer2:
            wnext_sb = load_const(wnext, [128, wnext.shape[1]], dt.bfloat16,
                                  "wnext")

        # gather groups, created lazily in consumption order
        gtiles = [{}, {}]

        def group_tile(h, g):
            if g in gtiles[h]:
                return gtiles[h][g]
            tot = plan.totA if h == 0 else plan.totB
            ck = min(GC, tot - g * GC)
            pool = gpa if h == 0 else gpb
            t = pool.tile([128, GC * F], dt.bfloat16)
            idx_sb = idxa_sb if h == 0 else idxb_sb
            half = tab[0:cfg.HALFROWS, :] if h == 0 else tab[cfg.HALFROWS:, :]
            nidx = ck * 128
            nc.gpsimd.dma_gather(
                out_ap=t[:, : ck * F].rearrange("p (c f) -> p c f", f=F),
                in_ap=half,
                idxs_ap=idx_sb[:, g * GC * 8: g * GC * 8 + ck * 8],
                num_idxs=nidx,
                num_idxs_reg=nidx,
                elem_size=F,
                queue_num=(h * 2 + g) % 4,
                single_packet=False,
            )
            gtiles[h][g] = t
            return t

        for t in range(cfg.NTILES):
            # chunk list for this tile in consumption order
            chunks = []
            for h, basearr in ((0, plan.abase), (1, plan.bbase)):
                for j in range(int(basearr[t]), int(basearr[t + 1])):
                    M = int(plan.baseM[h][1][j])
                    if M == 0:
                        continue
                    chunks.append((h, j, int(plan.baseM[h][0][j]), M,
                                   int(plan.slab_off[h][j])))

            # group bracketed by two half-bias matmuls so that start/stop
            # cover the full [0:128] region (sim zero-region discipline)
            ps = psmp.tile([128, F], dt.float32)
            nc.tensor.matmul(ps[:], ident_sb[:], bias_sb[:],
                             start=True, stop=False, skip_group_check=True)
            # dense self-loop term: scaled rows of this core's own shard
            ts_t = work.tile([128, F], dt.bfloat16, tag="ts")
            nc.scalar.dma_start(ts_t[:], tabself[t * 128:(t + 1) * 128, :])
            sc_t = work.tile([128, F], dt.bfloat16, tag="sc")
            nc.scalar.activation(sc_t[:], ts_t[:],
                                 mybir.ActivationFunctionType.Copy,
                                 scale=selfw_sb[:, t:t + 1])
            nc.tensor.matmul(ps[:], ident_sb[:], sc_t[:],
                             start=False, stop=False, skip_group_check=True)
            for h, j, b0, M, so in chunks:
                gt = group_tile(h, j // GC)
                slot = j % GC
                rhs = gt[:, slot * F:(slot + 1) * F]
                for s in range(3):
                    Mq = int(plan.pieces[h][j, s])
                    if Mq == 0:
                        continue
                    bs = (0, 32, 64)[s]
                    col = so + bs - b0
                    nc.tensor.matmul(
                        ps[bs:bs + Mq, :],
                        wsl_sb[:, col:col + Mq],
                        rhs,
                        start=False, stop=False,
                        skip_group_check=True,
                    )
            nc.tensor.matmul(ps[:], ident_sb[:], bias_sb[:],
                             start=False, stop=True, skip_group_check=True)

            # post-processing
            if layer2:
                act = work.tile([128, F], dt.bfloat16)
                nc.scalar.activation(act[:], ps[:],
                                     mybir.ActivationFunctionType.Relu)
                trp = pstr.tile([128, F], dt.bfloat16)
                for c in range(FCH):
                    nc.tensor.transpose(trp[:, c * 128:(c + 1) * 128],
                                        act[:, c * 128:(c + 1) * 128],
                                        ident_sb[:])
                actT = work.tile([128, F], dt.bfloat16)
                nc.vector.tensor_copy(actT[:], trp[:])

                ps2 = psmm.tile([128, cfg.OUT], dt.float32)
                for c in range(FCH):
                    nc.tensor.matmul(ps2[:], actT[:, c * 128:(c + 1) * 128],
                                     wnext_sb[:, c * cfg.OUT:(c + 1) * cfg.OUT],
                                     start=(c == 0), stop=(c == FCH - 1))
                o = work.tile([128, cfg.OUT], dt.bfloat16)
                nc.scalar.activation(o[:], ps2[:],
                                     mybir.ActivationFunctionType.Copy)
            else:
                o = work.tile([128, cfg.OUT], dt.float32)
                nc.scalar.activation(o[:], ps[:],
                                     mybir.ActivationFunctionType.Copy)
            nc.scalar.dma_start(out[t * 128:(t + 1) * 128, :], o[:])

    nc.finalize()
    return nc


# ---------------------------------------------------------------- host packing

def _pack_l1_inputs(cfg: Cfg, plan: Plan, x, W1):
    KCH = cfg.IN_DIM // 128
    w1r = np.zeros((128, KCH * cfg.HID), BF16)
    for c in range(KCH):
        w1r[:, c * cfg.HID:(c + 1) * cfg.HID] = W1[c * 128:(c + 1) * 128, :].astype(BF16)
    maps = []
    for k in range(cfg.NCORES):
        xs = np.zeros((cfg.NP, cfg.IN_DIM), np.float32)
        xs[:cfg.ND] = x[plan.nodes[k]]
        xtr = np.zeros((128, KCH * cfg.NP), BF16)
        for c in range(KCH):
            xtr[:, c * cfg.NP:(c + 1) * cfg.NP] = \
                xs[:, c * 128:(c + 1) * 128].T.astype(BF16)
        maps.append({"xt": xtr, "w1": w1r})
    return maps


def _pack_mp_inputs(cfg: Cfg, plan: Plan, table, Wn, b, layer2):
    F = cfg.HID if layer2 else cfg.OUT
    # the bias matmul runs twice per tile (group start + stop) -> send b/2
    biasr = np.tile((b * 0.5).astype(BF16)[None, :], (128, 1))
    ident = np.eye(128, dtype=BF16)
    maps = []
    for k in range(cfg.NCORES):
        ia, ib = plan.idxs[k]
        m = {
            "tab": table,
            "tabself": np.ascontiguousarray(
                table[k * cfg.NP:(k + 1) * cfg.NP]),
            "selfw": plan.selfw[k],
            "idxa": ia,
            "idxb": ib,
            "wsl": plan.wslab[k],
            "bias": biasr,
            "ident": ident,
        }
        if layer2:
            FCH = cfg.HID // 128
            wnr = np.zeros((128, FCH * cfg.OUT), BF16)
            for c in range(FCH):
                wnr[:, c * cfg.OUT:(c + 1) * cfg.OUT] = \
                    Wn[c * 128:(c + 1) * 128, :].astype(BF16)
            m["wnext"] = wnr
        maps.append(m)
    return maps


# ---------------------------------------------------------------- driver

def _run(nc, in_maps, cfg, trace=False):
    from concourse.bass_utils import run_bass_kernel_spmd
    res = run_bass_kernel_spmd(nc, in_maps, list(range(cfg.NCORES)), trace=trace)
    return res


def kernel_run(inputs, cfg=None, trace=False, sim=False):
    cfg = cfg or Cfg()
    x = np.asarray(inputs["x"], np.float32)
    plan = Plan(cfg, np.asarray(inputs["edge_index"]),
                np.asarray(inputs["edge_weight"], np.float32))
    W1 = np.asarray(inputs["W1"], np.float32)
    b1 = np.asarray(inputs["b1"], np.float32)
    W2 = np.asarray(inputs["W2"], np.float32)
    b2 = np.asarray(inputs["b2"], np.float32)
    Wp = np.asarray(inputs["Wp"], np.float32)
    bp = np.asarray(inputs["bp"], np.float32)

    results = []

    def run(build, maps, outname):
        nc = build()
        if sim:
            from concourse.bass_interp import CoreSim
            outs = []
            for k in range(cfg.NCORES):
                s = CoreSim(nc)
                for name, arr in maps[k].items():
                    s.tensor(name)[:] = arr
                s.simulate()
                outs.append({outname: s.tensor(outname).copy()})
            results.append(None)
            return outs
        r = _run(nc, maps, cfg, trace=trace)
        results.append(r)
        return r.results

    # fold the post-projection into layer 2: A(relu1@W2)@Wp = A(relu1@(W2@Wp))
    W2p = (W2 @ Wp).astype(np.float32)
    bpp = (b2 @ Wp + bp).astype(np.float32)

    r1 = run(lambda: _build_l1(cfg), _pack_l1_inputs(cfg, plan, x, W1), "h1")
    T1 = np.concatenate([np.asarray(r["h1"]).view(BF16) if r["h1"].dtype != BF16
                         else r["h1"] for r in r1], axis=0)

    r2 = run(lambda: _build_mp(cfg, plan, True),
             _pack_mp_inputs(cfg, plan, T1, W2p, b1, True), "out")
    T2 = np.concatenate([np.asarray(r["out"]).view(BF16)
                         if r["out"].dtype != BF16 else r["out"]
                         for r in r2], axis=0)

    r3 = run(lambda: _build_mp(cfg, plan, False),
             _pack_mp_inputs(cfg, plan, T2, None, bpp, False), "out")

    y = np.empty((cfg.N, cfg.OUT), np.float32)
    for k in range(cfg.NCORES):
        shard = np.asarray(r3[k]["out"], np.float32)
        y[plan.nodes[k]] = shard[:cfg.ND]
    return y, results


def kernel(**inputs):
    y, _ = kernel_run(inputs)
    return y

